# revision 13
# baseline (speedup 1.0000x reference)
"""Self-attention (1x1-conv QKV projections + NxN softmax attention + residual)
for x:(4,256,64,64) on 8 TRN2 NeuronCores.

Sharding: core = 2*b + h  ->  batch b in 0..3, query-half h in 0..1.
Each core computes out[b][:, h*2048:(h+1)*2048] (softmax is row-wise over
keys, so splitting query rows is embarrassingly parallel).

Per-core kernel (all matmuls float32r = PE fast-fp32 mode, 1 cycle/row):
  k_sb (32,4096) = Wk@x + bk, q_sb (32,2048) = Wq@x[:,msl] + bq
  v_sb (128,32,256): v^T tiles, v_T[n,c] = sum_c' x[c',n] WvT[c',c]
  energy (transposed, per key-tile pair): e[n,m] = sum_d k[d,n] q[d,m]
    -> (128,1024) PSUM pairs, double-buffered
  p = exp(e / sqrt(32))    (no max subtraction: |e*s| <~ 6, exp is <=2ulp)
  out[c,m] = sum_n v_T[n,c] p[n,m]   (K=128 full-array matmuls, PSUM-accum)
  rowsum[m] = sum_n p[n,m] via ones-lhsT matmuls accumulating on one bank
  final: out = out_raw / rowsum + bv + x_residual

k/q/v projections are interleaved per 512-column x-chunk so the PE starts
as soon as the first input DMA lands instead of waiting for all of x.
"""

import numpy as np

B, C, N = 4, 256, 4096
DK = 32
MH = N // 2          # 2048 query rows per core
NT = N // 128        # 32 key tiles
SBW = 512            # query superblock width
SCALE = 1.0 / float(np.sqrt(DK))

_cache = {}


def _build_nc():
    from contextlib import ExitStack

    import concourse.bacc as bacc
    import concourse.mybir as mybir
    import concourse.tile as tile

    f32 = mybir.dt.float32
    f32r = mybir.dt.float32r
    Exp = mybir.ActivationFunctionType.Exp
    add = mybir.AluOpType.add
    mult = mybir.AluOpType.mult

    nc = bacc.Bacc("TRN2", target_bir_lowering=False, debug=False)

    xf_d = nc.dram_tensor("xf", [C, N], f32r, kind="ExternalInput").ap()
    xq_d = nc.dram_tensor("xq", [C, MH], f32r, kind="ExternalInput").ap()
    wqt_d = nc.dram_tensor("wqt", [C, DK], f32r, kind="ExternalInput").ap()
    wkt_d = nc.dram_tensor("wkt", [C, DK], f32r, kind="ExternalInput").ap()
    wvt_d = nc.dram_tensor("wvt", [C, C], f32r, kind="ExternalInput").ap()
    bq_d = nc.dram_tensor("bq", [DK, 1], f32, kind="ExternalInput").ap()
    bk_d = nc.dram_tensor("bk", [DK, 1], f32, kind="ExternalInput").ap()
    bv_d = nc.dram_tensor("bv", [C, 1], f32, kind="ExternalInput").ap()
    ones_d = nc.dram_tensor("ones32", [128, DK], f32r, kind="ExternalInput").ap()
    out_d = nc.dram_tensor("out", [C, MH], f32, kind="ExternalOutput").ap()

    with tile.TileContext(nc) as tc, ExitStack() as ctx:
        const = ctx.enter_context(tc.tile_pool(name="const", bufs=1))

        # ---- weights / biases to SBUF ----
        wqt_sb = const.tile([128, 2, DK], f32r)
        wkt_sb = const.tile([128, 2, DK], f32r)
        wvt_sb = const.tile([128, 2, C], f32r)
        for a in range(2):
            nc.sync.dma_start(out=wqt_sb[:, a, :], in_=wqt_d[a * 128:(a + 1) * 128, :])
            nc.sync.dma_start(out=wkt_sb[:, a, :], in_=wkt_d[a * 128:(a + 1) * 128, :])

        bq_sb = const.tile([DK, 1], f32)
        bk_sb = const.tile([DK, 1], f32)
        bv_sb = const.tile([128, 2], f32)
        nc.sync.dma_start(out=bq_sb, in_=bq_d)
        nc.sync.dma_start(out=bk_sb, in_=bk_d)
        for a in range(2):
            nc.sync.dma_start(out=bv_sb[:, a:a + 1], in_=bv_d[a * 128:(a + 1) * 128, :])

        ones = const.tile([128, DK], f32r)
        nc.sync.dma_start(out=ones, in_=ones_d)

        # ---- inputs: xq first (q unblocks the main loop), then x chunks ----
        x_sb = const.tile([128, 2, N], f32r)
        xq_sb = const.tile([128, 2, MH], f32r)
        for j in range(4):
            for a in range(2):
                nc.sync.dma_start(
                    out=xq_sb[:, a, j * 512:(j + 1) * 512],
                    in_=xq_d[a * 128:(a + 1) * 128, j * 512:(j + 1) * 512])
        for j in range(8):
            for a in range(2):
                nc.sync.dma_start(
                    out=x_sb[:, a, j * 512:(j + 1) * 512],
                    in_=xf_d[a * 128:(a + 1) * 128, j * 512:(j + 1) * 512])

        for a in range(2):
            nc.sync.dma_start(out=wvt_sb[:, a, :], in_=wvt_d[a * 128:(a + 1) * 128, :])

        k_sb = const.tile([DK, N], f32r)
        q_sb = const.tile([DK, MH], f32r)
        v_sb = const.tile([128, NT, C], f32r)

        # ---- projections, interleaved per x-chunk ----
        with tc.tile_pool(name="proj_ps", bufs=2, space="PSUM") as pp:
            for j in range(MH // SBW):
                qp = pp.tile([DK, SBW], f32, name="qp", tag="qp")
                for a in range(2):
                    nc.tensor.matmul(
                        qp, wqt_sb[:, a, :],
                        xq_sb[:, a, j * SBW:(j + 1) * SBW],
                        start=(a == 0), stop=(a == 1))
                nc.vector.tensor_scalar_add(
                    q_sb[:, j * SBW:(j + 1) * SBW], qp, bq_sb)
            for j in range(N // SBW):
                kp = pp.tile([DK, SBW], f32, name="kp", tag="kp")
                for a in range(2):
                    nc.tensor.matmul(
                        kp, wkt_sb[:, a, :],
                        x_sb[:, a, j * SBW:(j + 1) * SBW],
                        start=(a == 0), stop=(a == 1))
                nc.vector.tensor_scalar_add(
                    k_sb[:, j * SBW:(j + 1) * SBW], kp, bk_sb)
                for t in range(4 * j, 4 * j + 4):
                    vp = pp.tile([128, C], f32, name="vp", tag="vp")
                    for a in range(2):
                        nc.tensor.matmul(
                            vp,
                            x_sb[:, a, t * 128:(t + 1) * 128],
                            wvt_sb[:, a, :],
                            start=(a == 0), stop=(a == 1))
                    nc.vector.tensor_copy(out=v_sb[:, t, :], in_=vp)

        # ---- main attention loop: 16 key-tile pairs per query superblock ----
        ep = ctx.enter_context(tc.tile_pool(name="e_ps", bufs=2, space="PSUM"))
        op = ctx.enter_context(tc.tile_pool(name="o_ps", bufs=1, space="PSUM"))
        rp = ctx.enter_context(tc.tile_pool(name="rs_ps", bufs=2, space="PSUM"))
        ppool = ctx.enter_context(tc.tile_pool(name="p_sb", bufs=3))
        misc = ctx.enter_context(tc.tile_pool(name="misc", bufs=2))
        outp = ctx.enter_context(tc.tile_pool(name="outp", bufs=2))

        for sbk in range(MH // SBW):
            msl = slice(sbk * SBW, (sbk + 1) * SBW)
            o_ps = [op.tile([128, SBW], f32, name=f"o_ps{c}", tag=f"o_ps{c}")
                    for c in range(2)]
            rs_ps = rp.tile([DK, SBW], f32)
            # 1-stage software pipeline: emit energy(pr) ahead of PV(pr-1)
            # so the PE FIFO never head-of-line blocks on exp(pr-1).
            pend = None
            for pr in range(NT // 2 + 1):
                if pr < NT // 2:
                    e_pair = ep.tile([128, 2 * SBW], f32,
                                     name="e_pair", tag="e_pair")
                    for i in range(2):
                        t = 2 * pr + i
                        nc.tensor.matmul(
                            e_pair[:, i * SBW:(i + 1) * SBW],
                            k_sb[:, t * 128:(t + 1) * 128],
                            q_sb[:, msl],
                            start=True, stop=True)
                if pend is not None:
                    p_prev, pr_prev = pend
                    for i in range(2):
                        t = 2 * pr_prev + i
                        prhs = p_prev[:, i * SBW:(i + 1) * SBW]
                        for c in range(2):
                            nc.tensor.matmul(
                                o_ps[c],
                                v_sb[:, t, c * 128:(c + 1) * 128],
                                prhs,
                                start=(t == 0), stop=(t == NT - 1))
                        nc.tensor.matmul(
                            rs_ps, ones, prhs,
                            start=(t == 0), stop=(t == NT - 1))
                if pr < NT // 2:
                    p_pair = ppool.tile([128, 2 * SBW], f32r,
                                        name="p_pair", tag="p_pair")
                    nc.scalar.activation(p_pair, e_pair, Exp, scale=SCALE)
                    pend = (p_pair, pr)

            # softmax denominator: partitions 0-31 of rs_ps all hold rowsum
            rec = misc.tile([1, SBW], f32)
            nc.vector.reciprocal(out=rec, in_=rs_ps[0:1, :])
            rec_rep = misc.tile([128, SBW], f32)
            nc.gpsimd.partition_broadcast(rec_rep, rec)

            for c in range(2):
                osb = outp.tile([128, SBW], f32, name=f"osb{c}", tag=f"osb{c}")
                nc.vector.scalar_tensor_tensor(
                    out=osb, in0=o_ps[c], scalar=0.0, in1=rec_rep,
                    op0=add, op1=mult)
                ofin = outp.tile([128, SBW], f32, name=f"ofin{c}", tag=f"ofin{c}")
                nc.vector.scalar_tensor_tensor(
                    out=ofin, in0=osb, scalar=bv_sb[:, c:c + 1],
                    in1=xq_sb[:, c, msl].bitcast(f32), op0=add, op1=add)
                nc.sync.dma_start(out=out_d[c * 128:(c + 1) * 128, msl], in_=ofin)

    nc.compile()
    return nc


def kernel(x, Wq, bq, Wk, bk, Wv, bv):
    from concourse import bass_utils

    x = np.asarray(x, np.float32)
    xf = np.ascontiguousarray(x.reshape(B, C, N))
    wqt = np.ascontiguousarray(np.asarray(Wq, np.float32).T)
    wkt = np.ascontiguousarray(np.asarray(Wk, np.float32).T)
    wvt = np.ascontiguousarray(np.asarray(Wv, np.float32).T)
    bq2 = np.ascontiguousarray(np.asarray(bq, np.float32).reshape(DK, 1))
    bk2 = np.ascontiguousarray(np.asarray(bk, np.float32).reshape(DK, 1))
    bv2 = np.ascontiguousarray(np.asarray(bv, np.float32).reshape(C, 1))
    ones32 = np.ones((128, DK), np.float32)

    if "nc" not in _cache:
        _cache["nc"] = _build_nc()
    nc = _cache["nc"]

    in_maps = []
    for core in range(8):
        b, h = core // 2, core % 2
        in_maps.append({
            "xf": xf[b],
            "xq": np.ascontiguousarray(xf[b][:, h * MH:(h + 1) * MH]),
            "wqt": wqt, "wkt": wkt, "wvt": wvt,
            "bq": bq2, "bk": bk2, "bv": bv2,
            "ones32": ones32,
        })

    res = bass_utils.run_bass_kernel_spmd(nc, in_maps, core_ids=list(range(8)))
    out = np.empty((B, C, N), np.float32)
    for core in range(8):
        b, h = core // 2, core % 2
        out[b][:, h * MH:(h + 1) * MH] = res.results[core]["out"]
    return out.reshape(B, C, 64, 64)


# revision 15
# speedup vs baseline: 12818.3509x; 12818.3509x over previous
"""Self-attention (1x1-conv QKV projections + NxN softmax attention + residual)
for x:(4,256,64,64) on 8 TRN2 NeuronCores.

Sharding: core = 2*b + h  ->  batch b in 0..3, query-half h in 0..1.
Each core computes out[b][:, h*2048:(h+1)*2048] (softmax is row-wise over
keys, so splitting query rows is embarrassingly parallel).

Per-core kernel (all matmuls float32r = PE fast-fp32 mode, 1 cycle/row):
  k_sb (32,4096) = Wk@x + bk, q_sb (32,2048) = Wq@x[:,msl] + bq
  v_sb (128,32,256): v^T tiles, v_T[n,c] = sum_c' x[c',n] WvT[c',c]
  energy (transposed, per key-tile pair): e[n,m] = sum_d k[d,n] q[d,m]
    -> (128,1024) PSUM pairs, double-buffered
  p = exp(e / sqrt(32))    (no max subtraction: |e*s| <~ 6, exp is <=2ulp)
  out[c,m] = sum_n v_T[n,c] p[n,m]   (K=128 full-array matmuls, PSUM-accum)
  rowsum[m] = sum_n p[n,m] via ones-lhsT matmuls accumulating on one bank
  final: out = out_raw / rowsum + bv + x_residual

k/q/v projections are interleaved per 512-column x-chunk so the PE starts
as soon as the first input DMA lands instead of waiting for all of x.
"""

import numpy as np

B, C, N = 4, 256, 4096
DK = 32
MH = N // 2          # 2048 query rows per core
NT = N // 128        # 32 key tiles
SBW = 512            # query superblock width
SCALE = 1.0 / float(np.sqrt(DK))

_cache = {}


def _build_nc():
    from contextlib import ExitStack

    import concourse.bacc as bacc
    import concourse.bass as bass
    import concourse.mybir as mybir
    import concourse.tile as tile

    f32 = mybir.dt.float32
    f32r = mybir.dt.float32r
    Exp = mybir.ActivationFunctionType.Exp
    add = mybir.AluOpType.add
    mult = mybir.AluOpType.mult

    nc = bacc.Bacc("TRN2", target_bir_lowering=False, debug=False)

    xf_d = nc.dram_tensor("xf", [C, N], f32r, kind="ExternalInput").ap()
    xq_d = nc.dram_tensor("xq", [C, MH], f32r, kind="ExternalInput").ap()
    wqt_d = nc.dram_tensor("wqt", [C, DK], f32r, kind="ExternalInput").ap()
    wkt_d = nc.dram_tensor("wkt", [C, DK], f32r, kind="ExternalInput").ap()
    wvt_d = nc.dram_tensor("wvt", [C, C], f32r, kind="ExternalInput").ap()
    bq_d = nc.dram_tensor("bq", [DK, 1], f32, kind="ExternalInput").ap()
    bk_d = nc.dram_tensor("bk", [DK, 1], f32, kind="ExternalInput").ap()
    bv_d = nc.dram_tensor("bv", [C, 1], f32, kind="ExternalInput").ap()
    ones_d = nc.dram_tensor("ones32", [128, DK], f32r, kind="ExternalInput").ap()
    out_d = nc.dram_tensor("out", [C, MH], f32, kind="ExternalOutput").ap()

    with tile.TileContext(nc) as tc, ExitStack() as ctx:
        const = ctx.enter_context(tc.tile_pool(name="const", bufs=1))

        # ---- weights / biases to SBUF ----
        wqt_sb = const.tile([128, 2, DK], f32r)
        wkt_sb = const.tile([128, 2, DK], f32r)
        wvt_sb = const.tile([128, 2, C], f32r)

        def split_c(dram_ap, width):
            # (256, width) -> stream (p, a, m) matching a [128, 2, width] tile
            return bass.AP(tensor=dram_ap.tensor, offset=dram_ap.offset,
                           ap=[[width, 128], [128 * width, 2], [1, width]])

        nc.sync.dma_start(out=wqt_sb, in_=split_c(wqt_d, DK))
        nc.sync.dma_start(out=wkt_sb, in_=split_c(wkt_d, DK))

        # ---- inputs: xq first (q unblocks the main loop), then x chunks ----
        x_sb = const.tile([128, 2, N], f32r)
        xq_sb = const.tile([128, 2, MH], f32r)

        def chunk_c(dram_ap, width, j, cw):
            return bass.AP(tensor=dram_ap.tensor, offset=dram_ap.offset + j * cw,
                           ap=[[width, 128], [128 * width, 2], [1, cw]])

        for j in range(4):
            nc.sync.dma_start(out=xq_sb[:, :, j * 512:(j + 1) * 512],
                              in_=chunk_c(xq_d, MH, j, 512))
        bq_sb = const.tile([DK, 1], f32)
        bk_sb = const.tile([DK, 1], f32)
        bv_sb = const.tile([128, 2], f32)
        nc.sync.dma_start(out=bq_sb, in_=bq_d)
        nc.sync.dma_start(out=bk_sb, in_=bk_d)
        for a in range(2):
            nc.sync.dma_start(out=bv_sb[:, a:a + 1], in_=bv_d[a * 128:(a + 1) * 128, :])

        ones = const.tile([128, DK], f32r)
        nc.sync.dma_start(out=ones, in_=ones_d)

        nc.sync.dma_start(out=wvt_sb, in_=split_c(wvt_d, C))
        for j in range(8):
            nc.sync.dma_start(out=x_sb[:, :, j * 512:(j + 1) * 512],
                              in_=chunk_c(xf_d, N, j, 512))

        k_sb = const.tile([DK, N], f32r)
        q_sb = const.tile([DK, MH], f32r)
        v_sb = const.tile([128, NT, C], f32r)

        # ---- projections, interleaved per x-chunk ----
        with tc.tile_pool(name="proj_ps", bufs=2, space="PSUM") as pp:
            for j in range(MH // SBW):
                qp = pp.tile([DK, SBW], f32, name="qp", tag="qp")
                for a in range(2):
                    nc.tensor.matmul(
                        qp, wqt_sb[:, a, :],
                        xq_sb[:, a, j * SBW:(j + 1) * SBW],
                        start=(a == 0), stop=(a == 1))
                nc.vector.tensor_scalar_add(
                    q_sb[:, j * SBW:(j + 1) * SBW], qp, bq_sb)
            for j in range(N // SBW):
                kp = pp.tile([DK, SBW], f32, name="kp", tag="kp")
                for a in range(2):
                    nc.tensor.matmul(
                        kp, wkt_sb[:, a, :],
                        x_sb[:, a, j * SBW:(j + 1) * SBW],
                        start=(a == 0), stop=(a == 1))
                nc.vector.tensor_scalar_add(
                    k_sb[:, j * SBW:(j + 1) * SBW], kp, bk_sb)
                for t in range(4 * j, 4 * j + 4):
                    vp = pp.tile([128, C], f32, name="vp", tag="vp")
                    for a in range(2):
                        nc.tensor.matmul(
                            vp,
                            x_sb[:, a, t * 128:(t + 1) * 128],
                            wvt_sb[:, a, :],
                            start=(a == 0), stop=(a == 1))
                    nc.vector.tensor_copy(out=v_sb[:, t, :], in_=vp)

        # ---- main attention loop: 16 key-tile pairs per query superblock ----
        ep = ctx.enter_context(tc.tile_pool(name="e_ps", bufs=2, space="PSUM"))
        op = ctx.enter_context(tc.tile_pool(name="o_ps", bufs=1, space="PSUM"))
        rp = ctx.enter_context(tc.tile_pool(name="rs_ps", bufs=2, space="PSUM"))
        ppool = ctx.enter_context(tc.tile_pool(name="p_sb", bufs=3))
        misc = ctx.enter_context(tc.tile_pool(name="misc", bufs=2))
        outp = ctx.enter_context(tc.tile_pool(name="outp", bufs=2))

        for sbk in range(MH // SBW):
            msl = slice(sbk * SBW, (sbk + 1) * SBW)
            o_ps = [op.tile([128, SBW], f32, name=f"o_ps{c}", tag=f"o_ps{c}")
                    for c in range(2)]
            rs_ps = rp.tile([DK, SBW], f32)
            # 1-stage software pipeline: emit energy(pr) ahead of PV(pr-1)
            # so the PE FIFO never head-of-line blocks on exp(pr-1).
            pend = None
            for pr in range(NT // 2 + 1):
                if pr < NT // 2:
                    e_pair = ep.tile([128, 2 * SBW], f32,
                                     name="e_pair", tag="e_pair")
                    for i in range(2):
                        t = 2 * pr + i
                        nc.tensor.matmul(
                            e_pair[:, i * SBW:(i + 1) * SBW],
                            k_sb[:, t * 128:(t + 1) * 128],
                            q_sb[:, msl],
                            start=True, stop=True)
                if pend is not None:
                    p_prev, pr_prev = pend
                    for i in range(2):
                        t = 2 * pr_prev + i
                        prhs = p_prev[:, i * SBW:(i + 1) * SBW]
                        for c in range(2):
                            nc.tensor.matmul(
                                o_ps[c],
                                v_sb[:, t, c * 128:(c + 1) * 128],
                                prhs,
                                start=(t == 0), stop=(t == NT - 1))
                        nc.tensor.matmul(
                            rs_ps, ones, prhs,
                            start=(t == 0), stop=(t == NT - 1))
                if pr < NT // 2:
                    p_pair = ppool.tile([128, 2 * SBW], f32r,
                                        name="p_pair", tag="p_pair")
                    nc.scalar.activation(p_pair, e_pair, Exp, scale=SCALE)
                    pend = (p_pair, pr)

            # softmax denominator: partitions 0-31 of rs_ps all hold rowsum
            rec = misc.tile([1, SBW], f32)
            nc.vector.reciprocal(out=rec, in_=rs_ps[0:1, :])
            rec_rep = misc.tile([128, SBW], f32)
            nc.gpsimd.partition_broadcast(rec_rep, rec)

            for c in range(2):
                osb = outp.tile([128, SBW], f32, name=f"osb{c}", tag=f"osb{c}")
                nc.vector.scalar_tensor_tensor(
                    out=osb, in0=o_ps[c], scalar=0.0, in1=rec_rep,
                    op0=add, op1=mult)
                ofin = outp.tile([128, SBW], f32, name=f"ofin{c}", tag=f"ofin{c}")
                nc.vector.scalar_tensor_tensor(
                    out=ofin, in0=osb, scalar=bv_sb[:, c:c + 1],
                    in1=xq_sb[:, c, msl].bitcast(f32), op0=add, op1=add)
                nc.sync.dma_start(out=out_d[c * 128:(c + 1) * 128, msl], in_=ofin)

    nc.compile()
    return nc


def kernel(x, Wq, bq, Wk, bk, Wv, bv):
    from concourse import bass_utils

    x = np.asarray(x, np.float32)
    xf = np.ascontiguousarray(x.reshape(B, C, N))
    wqt = np.ascontiguousarray(np.asarray(Wq, np.float32).T)
    wkt = np.ascontiguousarray(np.asarray(Wk, np.float32).T)
    wvt = np.ascontiguousarray(np.asarray(Wv, np.float32).T)
    bq2 = np.ascontiguousarray(np.asarray(bq, np.float32).reshape(DK, 1))
    bk2 = np.ascontiguousarray(np.asarray(bk, np.float32).reshape(DK, 1))
    bv2 = np.ascontiguousarray(np.asarray(bv, np.float32).reshape(C, 1))
    ones32 = np.ones((128, DK), np.float32)

    if "nc" not in _cache:
        _cache["nc"] = _build_nc()
    nc = _cache["nc"]

    in_maps = []
    for core in range(8):
        b, h = core // 2, core % 2
        in_maps.append({
            "xf": xf[b],
            "xq": np.ascontiguousarray(xf[b][:, h * MH:(h + 1) * MH]),
            "wqt": wqt, "wkt": wkt, "wvt": wvt,
            "bq": bq2, "bk": bk2, "bv": bv2,
            "ones32": ones32,
        })

    res = bass_utils.run_bass_kernel_spmd(nc, in_maps, core_ids=list(range(8)))
    out = np.empty((B, C, N), np.float32)
    for core in range(8):
        b, h = core // 2, core % 2
        out[b][:, h * MH:(h + 1) * MH] = res.results[core]["out"]
    return out.reshape(B, C, 64, 64)
